# revision 1
# baseline (speedup 1.0000x reference)
"""Trainium2 Bass kernel v3 for link-prediction MLP (dense_mlp / ridge).

Same math/sharding as the baseline kernel (16 (src-quarter, dst-quarter)
buckets, 2 per core; Q7 dma_gather with int16 local indices), with:
  - 4 SWDGE queues, gathers alternating across queues (HW: ~4x gather rate),
  - table kept fp32 (512B descriptors; sub-512B descs run 2x slower on HW),
  - 2-group partition stacking for the MLP: relu/activations work on
    [64, 512] tiles, final layer as one block-diagonal matmul -> [2, 512],
  - transposed features evacuated psum->sbuf by the ACT engine in [128,1024]
    chunks; DVE does the edge-feature product and pred bias add.

Per core, per 4096-edge super-tile:
  2x dma_gather (fp32, edge-major) -> DVE mul -> fp16 feat -> 32 PE
  transposes (4 psum banks of 8) -> 4 ACT evacs -> per 1024-edge pair:
  2x mm1 into [64,512] psum, ACT relu, 2x mm2, ACT relu, 1x mm3
  (block-diag W3) -> [2,512] psum -> DVE +b3 into [2,2048] stage -> DMA out.
"""

import math

import numpy as np

import concourse.bass as bass
import concourse.mybir as mybir
import concourse.tile as tile
from concourse import bacc
from concourse.bass_utils import run_bass_kernel_spmd
from concourse.masks import make_identity

P = 128
SUP = 4096
CH = SUP // P      # 32 chunks of 128 edges
N_CORES = 8
EMB_DIM = 128
NUM_NODES = 100000
NQ = 4
QS = NUM_NODES // NQ  # 25000 rows per quarter
H = 32

F16 = mybir.dt.float16
F32 = mybir.dt.float32
I16 = mybir.dt.int16

# bucket pairing: each core's two buckets share the same two quarters
PAIRS = [
    (0, 1), (1, 0),   # core 0
    (2, 3), (3, 2),   # core 1
    (0, 2), (2, 0),   # core 2
    (1, 3), (3, 1),   # core 3
    (0, 3), (3, 0),   # core 4
    (1, 2), (2, 1),   # core 5
    (0, 0), (1, 1),   # core 6
    (2, 2), (3, 3),   # core 7
]

relu = mybir.ActivationFunctionType.Relu
act_copy = mybir.ActivationFunctionType.Copy


def build_program(nt: int, passes: int = 1, mode: str = "full_pre",
                 gbufs: int = 3, single_packet: bool = False):
    do_gather = "conly" != mode
    do_mlp = mode in ("full", "full_pre", "conly")
    pre_idx = "pre" in mode
    one_tbl = mode == "gonly_pre1"
    nc = bacc.Bacc(None, target_bir_lowering=False, num_swdge_queues=4)

    qsrc = [
        nc.dram_tensor(f"qsrc{i}", [QS, EMB_DIM], F32, kind="ExternalInput")
        for i in range(2)
    ]
    qdst = [
        nc.dram_tensor(f"qdst{i}", [QS, EMB_DIM], F32, kind="ExternalInput")
        for i in range(2)
    ]
    isrc = nc.dram_tensor("isrc", [2 * nt, P, SUP // 16], I16, kind="ExternalInput")
    idst = nc.dram_tensor("idst", [2 * nt, P, SUP // 16], I16, kind="ExternalInput")
    w1 = nc.dram_tensor("w1", [EMB_DIM, H], F16, kind="ExternalInput")
    w2r = nc.dram_tensor("w2r", [2 * H, H], F16, kind="ExternalInput")
    w3b = nc.dram_tensor("w3b", [2 * H, 2], F16, kind="ExternalInput")
    b1r = nc.dram_tensor("b1r", [2 * H, 1], F32, kind="ExternalInput")
    b2r = nc.dram_tensor("b2r", [2 * H, 1], F32, kind="ExternalInput")
    b3r = nc.dram_tensor("b3r", [2, 1], F32, kind="ExternalInput")
    out = nc.dram_tensor("out", [2 * nt, 2, SUP // 2], F32, kind="ExternalOutput")

    with tile.TileContext(nc) as tc:
        with (
            tc.tile_pool(name="const", bufs=1) as cpool,
            tc.tile_pool(name="gpool", bufs=gbufs) as gpool,
            tc.tile_pool(name="fpool", bufs=2) as fpool,
            tc.tile_pool(name="ffm", bufs=3) as ffm_pool,
            tc.tile_pool(name="hpool", bufs=3) as hpool,
            tc.tile_pool(name="ppool", bufs=2) as ppool,
            tc.tile_pool(name="idxp", bufs=3) as idxp,
            tc.tile_pool(name="psT", bufs=2, space="PSUM") as psT,
            tc.tile_pool(name="ps1", bufs=2, space="PSUM") as ps1,
            tc.tile_pool(name="ps2", bufs=2, space="PSUM") as ps2,
            tc.tile_pool(name="psP", bufs=2, space="PSUM") as psP,
        ):
            ident = cpool.tile([P, P], F16)
            make_identity(nc, ident[:])
            w1s = cpool.tile([EMB_DIM, H], F16)
            nc.sync.dma_start(w1s[:], w1[:])
            w2s = cpool.tile([2 * H, H], F16)
            nc.sync.dma_start(w2s[:], w2r[:])
            w3s = cpool.tile([2 * H, 2], F16)
            nc.sync.dma_start(w3s[:], w3b[:])
            b1s = cpool.tile([2 * H, 1], F32)
            nc.sync.dma_start(b1s[:], b1r[:])
            b2s = cpool.tile([2 * H, 1], F32)
            nc.sync.dma_start(b2s[:], b2r[:])
            b3s = cpool.tile([2, 1], F32)
            nc.sync.dma_start(b3s[:], b3r[:])
            gz = dz = None
            if not do_gather:
                gz = cpool.tile([P, CH * EMB_DIM], F32)
                nc.vector.memset(gz[:], 0.5)
                dz = cpool.tile([P, CH * EMB_DIM], F32)
                nc.vector.memset(dz[:], 0.5)
            sall = dall = None
            if pre_idx:
                S16 = SUP // 16
                sall = cpool.tile([P, 2 * nt * S16], I16)
                dall = cpool.tile([P, 2 * nt * S16], I16)
                for tt2 in range(2 * nt):
                    nc.sync.dma_start(sall[:, tt2 * S16 : (tt2 + 1) * S16], isrc[tt2])
                    nc.sync.dma_start(dall[:, tt2 * S16 : (tt2 + 1) * S16], idst[tt2])

            for _ in range(passes):
              for pair in range(2):
                  qs_t, qd_t = qsrc[pair], qdst[pair]
                  if one_tbl:
                      qs_t = qd_t = qsrc[0]
                  for t in range(nt):
                      tt = pair * nt + t
                      if pre_idx:
                          S16 = SUP // 16
                          sidx = sall[:, tt * S16 : (tt + 1) * S16]
                          didx = dall[:, tt * S16 : (tt + 1) * S16]
                      else:
                          sidx = idxp.tile([P, SUP // 16], I16, tag="sidx")
                          nc.sync.dma_start(sidx[:], isrc[tt])
                          didx = idxp.tile([P, SUP // 16], I16, tag="didx")
                          nc.sync.dma_start(didx[:], idst[tt])

                      # gather slot i -> partition i%128, block i//128
                      if do_gather:
                          gs = gpool.tile([P, CH * EMB_DIM], F32, tag="gs")
                          nc.gpsimd.dma_gather(
                              gs[:].rearrange("p (b f) -> p b f", b=CH),
                              qs_t[:], sidx[:], SUP, SUP, EMB_DIM,
                              single_packet=single_packet,
                              queue_num=(2 * tt) % 4,
                          )
                          gd = gpool.tile([P, CH * EMB_DIM], F32, tag="gd")
                          nc.gpsimd.dma_gather(
                              gd[:].rearrange("p (b f) -> p b f", b=CH),
                              qd_t[:], didx[:], SUP, SUP, EMB_DIM,
                              single_packet=single_packet,
                              queue_num=(2 * tt + 1) % 4,
                          )
                      else:
                          gs, gd = gz, dz
                      if not do_mlp:
                          continue

                      # feat[p, c*128+f] = src*dst of edge slot (c*128+p)
                      feat = fpool.tile([P, SUP], F16)
                      nc.vector.tensor_mul(feat[:], gs[:], gd[:])

                      preds = ppool.tile([2, SUP // 2], F32, tag="preds")
                      for j in range(4):  # 1024-edge pairs
                          ftp = psT.tile([P, 1024], F16, tag="ftp")
                          for cc in range(8):
                              c = j * 8 + cc
                              nc.tensor.transpose(
                                  ftp[:, cc * P : (cc + 1) * P],
                                  feat[:, c * P : (c + 1) * P],
                                  ident[:],
                              )
                          ffm = ffm_pool.tile([P, 1024], F16)
                          nc.scalar.activation(ffm[:], ftp[:], act_copy)

                          h1p = ps1.tile([2 * H, 512], F32, tag="h1p")
                          for g in range(2):
                              nc.tensor.matmul(
                                  h1p[g * H : (g + 1) * H, :], lhsT=w1s[:],
                                  rhs=ffm[:, g * 512 : (g + 1) * 512],
                                  start=True, stop=True,
                              )
                          h1s = hpool.tile([2 * H, 512], F16, tag="h1s")
                          nc.scalar.activation(h1s[:], h1p[:], relu, bias=b1s[:])
                          h2p = ps2.tile([2 * H, 512], F32, tag="h2p")
                          for g in range(2):
                              nc.tensor.matmul(
                                  h2p[g * H : (g + 1) * H, :],
                                  lhsT=w2s[g * H : (g + 1) * H, :],
                                  rhs=h1s[g * H : (g + 1) * H, :],
                                  start=True, stop=True,
                              )
                          h2s = hpool.tile([2 * H, 512], F16, tag="h2s")
                          nc.scalar.activation(h2s[:], h2p[:], relu, bias=b2s[:])
                          predp = psP.tile([2, 512], F32, tag="predp")
                          nc.tensor.matmul(
                              predp[:], lhsT=w3s[:], rhs=h2s[:],
                              start=True, stop=True,
                          )
                          nc.vector.tensor_add(
                              preds[:, j * 512 : (j + 1) * 512], predp[:],
                              b3s[:].to_broadcast([2, 512]),
                          )
                      nc.sync.dma_start(out[tt], preds[:])

    nc.finalize()
    return nc


_PROGRAM_CACHE: dict[tuple, object] = {}


def _get_program(nt: int, passes: int = 1):
    key = (nt, passes)
    if key not in _PROGRAM_CACHE:
        _PROGRAM_CACHE[key] = build_program(nt, passes)
    return _PROGRAM_CACHE[key]


def _wrap_idx(local_ids, nt):
    """[nt*SUP] int16 -> [nt, 128, SUP//16] wrapped (idx k -> [k%16, k//16]),
    replicated 8x across partitions."""
    w = local_ids.reshape(nt, SUP // 16, 16).transpose(0, 2, 1)
    return np.ascontiguousarray(np.tile(w, (1, 8, 1)))


# edge slot for each flat (row, col) of the [2, 2048] per-supertile output:
# slot = 1024*(col//512) + 512*row + col%512
_PERM = np.empty(SUP, dtype=np.int64)
for _row in range(2):
    for _col in range(SUP // 2):
        _PERM[1024 * (_col // 512) + 512 * _row + _col % 512] = (
            _row * (SUP // 2) + _col
        )


def prepare(node_id, edge_label_index, emb_table, W1, b1, W2, b2, W3, b3):
    """Host-side sharding: bucket edges, build per-core input maps."""
    node_id = np.asarray(node_id)
    edge_label_index = np.asarray(edge_label_index)
    emb_table = np.ascontiguousarray(np.asarray(emb_table, dtype=np.float32))

    E = edge_label_index.shape[1]
    src_all = np.asarray(node_id[edge_label_index[0]], dtype=np.int64)
    dst_all = np.asarray(node_id[edge_label_index[1]], dtype=np.int64)

    bucket = (src_all // QS) * NQ + (dst_all // QS)
    border = {}
    for k, (s, d) in enumerate(PAIRS):
        border[s * NQ + d] = k
    bucket_slot = np.asarray([border[b] for b in range(NQ * NQ)])[bucket]
    order = np.argsort(bucket_slot, kind="stable")
    counts = np.bincount(bucket_slot, minlength=NQ * NQ)
    starts = np.zeros(NQ * NQ + 1, dtype=np.int64)
    np.cumsum(counts, out=starts[1:])

    nt = max(1, math.ceil(int(counts.max()) / SUP))
    cap = nt * SUP

    quarters = [emb_table[i * QS : (i + 1) * QS] for i in range(NQ)]
    w1h = np.ascontiguousarray(np.asarray(W1, dtype=np.float16))
    w2h = np.asarray(W2, dtype=np.float16)
    w3h = np.asarray(W3, dtype=np.float16).reshape(H)
    w2r = np.ascontiguousarray(np.tile(w2h, (2, 1)))
    w3b = np.zeros((2 * H, 2), np.float16)
    for g in range(2):
        w3b[g * H : (g + 1) * H, g] = w3h
    b1c = np.asarray(b1, dtype=np.float32).reshape(H, 1)
    b2c = np.asarray(b2, dtype=np.float32).reshape(H, 1)
    b1rep = np.ascontiguousarray(np.tile(b1c, (2, 1)))
    b2rep = np.ascontiguousarray(np.tile(b2c, (2, 1)))
    b3c = np.ascontiguousarray(
        np.broadcast_to(np.asarray(b3, dtype=np.float32).reshape(1, 1), (2, 1)).copy()
    )

    in_maps = []
    edge_pos = []
    for k in range(N_CORES):
        m = {"w1": w1h, "w2r": w2r, "w3b": w3b, "b1r": b1rep, "b2r": b2rep,
             "b3r": b3c}
        isrc = np.zeros((2, cap), dtype=np.int16)
        idst = np.zeros((2, cap), dtype=np.int16)
        pos_pair = []
        for j in range(2):
            bq = 2 * k + j
            s_q, d_q = PAIRS[bq]
            pos = order[starts[bq] : starts[bq + 1]]
            pos_pair.append(pos)
            isrc[j, : len(pos)] = (src_all[pos] - s_q * QS).astype(np.int16)
            idst[j, : len(pos)] = (dst_all[pos] - d_q * QS).astype(np.int16)
            m[f"qsrc{j}"] = quarters[s_q]
            m[f"qdst{j}"] = quarters[d_q]
        m["isrc"] = _wrap_idx(isrc.reshape(2 * nt, SUP), 2 * nt).reshape(
            2 * nt, P, SUP // 16
        )
        m["idst"] = _wrap_idx(idst.reshape(2 * nt, SUP), 2 * nt).reshape(
            2 * nt, P, SUP // 16
        )
        edge_pos.append(pos_pair)
        in_maps.append(m)

    return {"in_maps": in_maps, "edge_pos": edge_pos, "nt": nt, "E": E}


def unshard(prep, results):
    """results: list of per-core {"out": [2*nt, 2, 2048]} -> full [E] preds."""
    nt = prep["nt"]
    preds = np.empty(prep["E"], dtype=np.float32)
    for k in range(N_CORES):
        o = results[k]["out"]
        for j in range(2):
            pos = prep["edge_pos"][k][j]
            flat = o[j * nt : (j + 1) * nt].reshape(nt, SUP)[:, _PERM].reshape(-1)
            preds[pos] = flat[: len(pos)]
    return preds


def kernel(node_id, edge_label_index, emb_table, W1, b1, W2, b2, W3, b3):
    prep = prepare(node_id, edge_label_index, emb_table, W1, b1, W2, b2, W3, b3)
    nc = _get_program(prep["nt"])
    res = run_bass_kernel_spmd(
        nc, prep["in_maps"], core_ids=list(range(N_CORES)), trace=False
    )
    return unshard(prep, res.results)

